# revision 4
# baseline (speedup 1.0000x reference)
"""Trainium2 Bass kernel for GaussianScene2 (3D gaussian splatting renderer).

Data-parallel over image row-bands: each of 8 cores composites one 16-row
band (2048 px). Per-gaussian projection (EWA 2D covariance inverse, pixel
means, radius, log-sigmoid opacity) is precomputed on host in f64, packed
into 10 block-major planes, and sharded across the cores: each core ships
only 1/8th of the depth-sorted list (~21KB) and an on-device AllGather over
NeuronLink reconstructs the full list, cutting host->device traffic over
the PJRT tunnel ~8x. On device, per-block [128 gaussians x 2048 px] tiles
evaluate the 2D gaussians; alpha converts to log-transmittance, and the
front-to-back compositing cumsum runs as triangular matmuls on the PE
engine with the across-block carry accumulated in PSUM; colors accumulate
via a second matmul into a [3, 2048] PSUM image. Pixel ramps and triangular
masks are iota-generated on device. The PJRT executable wrapper is built
once and cached (run_bass_kernel_spmd would retrace it every call, ~200ms),
and the donated output zero-buffers are pre-staged on device between calls
so the timed path pays only input upload + dispatch + execute + fetch.
"""

import sys

sys.path.insert(0, "/opt/trn_rl_repo")

import numpy as np

H = 128
W = 128
NCORES = 8
ROWS = H // NCORES          # rows per core
NPIX = ROWS * W             # pixels per core
CHUNK = 512                 # psum bank free size (fp32)
NCH = NPIX // CHUNK
ZNEAR = 0.2
MIN_T = 0.01
BIGNEG = 1.0e30

_program_cache = {}


def _build_program(nb, use_clamp):
    """nb = TOTAL gaussian blocks (multiple of NCORES); each core ships nb/8."""
    from contextlib import ExitStack

    import concourse.bacc as bacc
    import concourse.tile as tile
    from concourse import mybir

    F32 = mybir.dt.float32
    AF = mybir.ActivationFunctionType
    ALU = mybir.AluOpType
    LNMINT = float(np.log(np.float32(MIN_T)))

    assert nb % NCORES == 0
    nbs = nb // NCORES              # blocks per shard
    CSH = 10 * nbs + ROWS           # per-core input cols
    CG = 10 * nbs                   # gathered cols per shard

    nc = bacc.Bacc("TRN2", target_bir_lowering=False, debug=False)

    pk_d = nc.dram_tensor("pk", [128, CSH], F32, kind="ExternalInput")
    img_d = nc.dram_tensor("img", [3, NPIX], F32, kind="ExternalOutput")
    gin = nc.dram_tensor("gin", [128, CG], F32)
    gout = nc.dram_tensor("gout", [NCORES, 128, CG], F32, addr_space="Shared")

    with tile.TileContext(nc) as tc, ExitStack() as ctx:
        P = ctx.enter_context(tc.tile_pool(name="pre", bufs=1))
        WK = ctx.enter_context(tc.tile_pool(name="work", bufs=2))
        PS = ctx.enter_context(tc.tile_pool(name="psum", bufs=1, space="PSUM"))

        def pt(shape, tag):
            return P.tile(shape, F32, tag=tag, name=tag)

        # ---- shard in, AllGather, unpack to SBUF ----
        nc.sync.dma_start(gin[:], pk_d[:, :CG])
        nc.gpsimd.collective_compute(
            "AllGather", ALU.bypass, replica_groups=[list(range(NCORES))],
            ins=[gin[:]], outs=[gout[:]])

        pl = pt([128, 7, nb], "pl")          # 7 planes x all blocks
        colT = pt([128, 3 * nb], "colT")     # interleaved colors per block
        for s in range(NCORES):
            src = gout[s]                    # [128, CG]
            nc.sync.dma_start(
                pl[:, :, s * nbs:(s + 1) * nbs],
                src[:, :7 * nbs].rearrange("p (t n) -> p t n", t=7))
            nc.sync.dma_start(
                colT[:, 3 * nbs * s:3 * nbs * (s + 1)],
                src[:, 7 * nbs:])
        rowg_t = pt([128, ROWS], "rowg")
        nc.sync.dma_start(rowg_t[:], pk_d[:, CG:])

        px = pl[:, 0, :]
        py = pl[:, 1, :]
        m05ia = pl[:, 2, :]
        m05ic = pl[:, 3, :]
        mib = pl[:, 4, :]
        rad = pl[:, 5, :]
        lsigm = pl[:, 6, :]
        rowg = rowg_t[:]

        ts_ = nc.vector.tensor_scalar
        ttv = nc.vector.tensor_tensor
        ttp = nc.gpsimd.tensor_tensor
        act = nc.scalar.activation

        # ---- on-device constants: pixel-x ramp, row index, triangular masks
        gx = pt([128, 128], "gx")
        nc.gpsimd.iota(gx[:], [[1, 128]], channel_multiplier=0,
                       allow_small_or_imprecise_dtypes=True)
        rix = pt([128, 128], "rix")
        nc.gpsimd.iota(rix[:], [[0, 128]], channel_multiplier=1,
                       allow_small_or_imprecise_dtypes=True)
        tris = pt([128, 128], "tris")
        ttv(out=tris[:], in0=rix[:], in1=gx[:], op=ALU.is_le)
        lows = pt([128, 128], "lows")
        ttv(out=lows[:], in0=rix[:], in1=gx[:], op=ALU.is_gt)

        # ---- per-block pixel-x precompute: qxm[g, b, w], bxw[g, b, w] ----
        qxm = pt([128, nb, 128], "qxm")
        bxw = pt([128, nb, 128], "bxw")
        dxw = WK.tile([128, nb, 128], F32, tag="dxw", name="dxw")
        tmpx = WK.tile([128, nb, 128], F32, tag="tmpx", name="tmpx")
        gx_b = gx[:].unsqueeze(1).broadcast_to([128, nb, 128])
        px_b = px.unsqueeze(2).broadcast_to([128, nb, 128])
        rad_b = rad.unsqueeze(2).broadcast_to([128, nb, 128])
        ttp(out=dxw[:], in0=gx_b, in1=px_b, op=ALU.subtract)
        act(out=tmpx[:], in_=dxw[:], func=AF.Abs)
        ttv(out=tmpx[:], in0=tmpx[:], in1=rad_b, op=ALU.is_le)
        ts_(out=tmpx[:], in0=tmpx[:], scalar1=BIGNEG, scalar2=BIGNEG, op0=ALU.mult, op1=ALU.subtract)
        m05ia_b = m05ia.unsqueeze(2).broadcast_to([128, nb, 128])
        ttp(out=qxm[:], in0=dxw[:], in1=dxw[:], op=ALU.mult)
        ttp(out=qxm[:], in0=qxm[:], in1=m05ia_b, op=ALU.mult)
        ttp(out=qxm[:], in0=qxm[:], in1=tmpx[:], op=ALU.add)
        mib_b = mib.unsqueeze(2).broadcast_to([128, nb, 128])
        ttp(out=bxw[:], in0=dxw[:], in1=mib_b, op=ALU.mult)

        # ---- per-block row precompute: dyr[g, b, r], sylm[g, b, r] ----
        dyr = pt([128, nb, ROWS], "dyr")
        sylm = pt([128, nb, ROWS], "sylm")
        tmpy = WK.tile([128, nb, ROWS], F32, tag="tmpy", name="tmpy")
        rowg_b = rowg.unsqueeze(1).broadcast_to([128, nb, ROWS])
        py_b = py.unsqueeze(2).broadcast_to([128, nb, ROWS])
        radr_b = rad.unsqueeze(2).broadcast_to([128, nb, ROWS])
        m05ic_b = m05ic.unsqueeze(2).broadcast_to([128, nb, ROWS])
        ttp(out=dyr[:], in0=rowg_b, in1=py_b, op=ALU.subtract)
        act(out=tmpy[:], in_=dyr[:], func=AF.Abs)
        ttv(out=tmpy[:], in0=tmpy[:], in1=radr_b, op=ALU.is_le)
        ts_(out=tmpy[:], in0=tmpy[:], scalar1=BIGNEG, scalar2=BIGNEG, op0=ALU.mult, op1=ALU.subtract)
        ttp(out=sylm[:], in0=dyr[:], in1=dyr[:], op=ALU.mult)
        ttp(out=sylm[:], in0=sylm[:], in1=m05ic_b, op=ALU.mult)
        ttp(out=sylm[:], in0=sylm[:], in1=tmpy[:], op=ALU.add)

        # ---- main compositing loop over gaussian blocks ----
        psS = PS.tile([128, NPIX], F32, tag="psS", name="psS")
        psI = PS.tile([3, NPIX], F32, tag="psI", name="psI")

        for b in range(nb):
            power = WK.tile([128, ROWS, 128], F32, tag="power", name="power")
            bx_b = bxw[:, b, :].unsqueeze(1).broadcast_to([128, ROWS, 128])
            dy_b = dyr[:, b, :].unsqueeze(2).broadcast_to([128, ROWS, 128])
            qx_b = qxm[:, b, :].unsqueeze(1).broadcast_to([128, ROWS, 128])
            sy_b = sylm[:, b, :].unsqueeze(2).broadcast_to([128, ROWS, 128])
            ttp(out=power[:], in0=bx_b, in1=dy_b, op=ALU.mult)
            ttp(out=power[:], in0=power[:], in1=qx_b, op=ALU.add)
            ttv(out=power[:], in0=power[:], in1=sy_b, op=ALU.add)
            pw = power[:].rearrange("g r w -> g (r w)")
            ls_b = lsigm[:, b:b + 1]
            ts_(out=pw, in0=pw, scalar1=ls_b, scalar2=ls_b, op0=ALU.add, op1=ALU.min)
            alpha = WK.tile([128, NPIX], F32, tag="alpha", name="alpha")
            act(out=alpha[:], in_=pw, func=AF.Exp)
            if use_clamp:
                ts_(out=alpha[:], in0=alpha[:], scalar1=0.99, scalar2=None, op0=ALU.min)
            lt = WK.tile([128, NPIX], F32, tag="lt", name="lt")
            act(out=lt[:], in_=alpha[:], func=AF.Ln, scale=-1.0, bias=1.0)

            for k in range(NCH):
                sl = slice(k * CHUNK, (k + 1) * CHUNK)
                nc.tensor.matmul(out=psS[:, sl], lhsT=tris[:],
                                 rhs=lt[:, sl],
                                 start=(b == 0), stop=True,
                                 skip_group_check=(b != 0))

            sprev = WK.tile([128, NPIX], F32, tag="power", name="sprev")
            maskt = WK.tile([128, NPIX], F32, tag="alpha", name="alpha")
            for k in range(NCH):
                sl = slice(k * CHUNK, (k + 1) * CHUNK)
                ttv(out=sprev[:, sl], in0=psS[:, sl], in1=lt[:, sl], op=ALU.subtract)
                ts_(out=maskt[:, sl], in0=psS[:, sl], scalar1=LNMINT, scalar2=None,
                    op0=ALU.is_ge)
            tprev = WK.tile([128, NPIX], F32, tag="lt", name="lt")
            act(out=tprev[:], in_=sprev[:], func=AF.Exp)
            contrib = WK.tile([128, NPIX], F32, tag="contrib", name="contrib")
            nc.gpsimd.tensor_tensor(out=contrib[:], in0=tprev[:], in1=alpha[:], op=ALU.mult)
            half = NPIX // 2
            ttp(out=contrib[:, :half], in0=contrib[:, :half],
                in1=maskt[:, :half], op=ALU.mult)
            nc.gpsimd.tensor_tensor(out=contrib[:, half:], in0=contrib[:, half:],
                                    in1=maskt[:, half:], op=ALU.mult)

            for k in range(NCH):
                sl = slice(k * CHUNK, (k + 1) * CHUNK)
                nc.tensor.matmul(out=psI[:, sl],
                                 lhsT=colT[:, 3 * b:3 * b + 3],
                                 rhs=contrib[:, sl],
                                 start=(b == 0), stop=True,
                                 skip_group_check=(b != 0))

            if b != nb - 1:
                for k in range(NCH):
                    sl = slice(k * CHUNK, (k + 1) * CHUNK)
                    nc.tensor.matmul(out=psS[:, sl], lhsT=lows[:],
                                     rhs=lt[:, sl],
                                     start=False, stop=True, skip_group_check=True)

        imgsb = P.tile([3, NPIX], F32, tag="imgsb", name="imgsb")
        for k in range(NCH):
            sl = slice(k * CHUNK, (k + 1) * CHUNK)
            nc.vector.tensor_copy(out=imgsb[:, sl], in_=psI[:, sl])
        nc.sync.dma_start(img_d[:], imgsb[:])

    nc.compile()
    return nc


def _make_runner(nc, n_cores=NCORES):
    import jax
    from jax.sharding import Mesh, PartitionSpec
    from jax.experimental.shard_map import shard_map

    from concourse import mybir
    from concourse.bass2jax import (_bass_exec_p, install_neuronx_cc_hook,
                                    partition_id_tensor)

    install_neuronx_cc_hook()
    pn = nc.partition_id_tensor.name if nc.partition_id_tensor else None
    in_names, out_names, out_avals, zero_outs = [], [], [], []
    for alloc in nc.m.functions[0].allocations:
        if not isinstance(alloc, mybir.MemoryLocationSet):
            continue
        name = alloc.memorylocations[0].name
        if alloc.kind == "ExternalInput":
            if name != pn:
                in_names.append(name)
        elif alloc.kind == "ExternalOutput":
            shape = tuple(alloc.tensor_shape)
            dtype = mybir.dt.np(alloc.dtype)
            out_names.append(name)
            out_avals.append(jax.core.ShapedArray(shape, dtype))
            zero_outs.append(np.zeros(shape, dtype))
    n_params = len(in_names)
    n_outs = len(out_avals)
    in_all = in_names + out_names + ([pn] if pn else [])
    donate = tuple(range(n_params, n_params + n_outs))

    def _body(*args):
        ops = list(args)
        if pn is not None:
            ops.append(partition_id_tensor())
        return tuple(_bass_exec_p.bind(
            *ops, out_avals=tuple(out_avals), in_names=tuple(in_all),
            out_names=tuple(out_names), lowering_input_output_aliases=(),
            sim_require_finite=True, sim_require_nnan=True, nc=nc))

    mesh = Mesh(np.asarray(jax.devices()[:n_cores]), ("core",))
    fn = jax.jit(
        shard_map(_body, mesh=mesh,
                  in_specs=(PartitionSpec("core"),) * (n_params + n_outs),
                  out_specs=(PartitionSpec("core"),) * len(out_names),
                  check_rep=False),
        donate_argnums=donate, keep_unused=True)

    # The zero output-buffers are a PJRT output-binding artifact (the NEFF
    # writes every element of img). Pre-stage them on device between calls
    # so the timed path never uploads them; donation consumes one set per
    # call, so schedule the next device_put right after each run.
    from jax.sharding import NamedSharding
    zsharding = NamedSharding(mesh, PartitionSpec("core"))

    def _stage_zeros():
        return [
            jax.device_put(
                np.zeros((n_cores * z.shape[0], *z.shape[1:]), z.dtype),
                zsharding)
            for z in zero_outs
        ]

    state = {"zeros": _stage_zeros()}

    def run(in_maps):
        concat_in = [
            np.concatenate([np.asarray(m[name]) for m in in_maps], axis=0)
            for name in in_names
        ]
        concat_zeros = state["zeros"]
        out_arrs = fn(*concat_in, *concat_zeros)
        results = [
            {name: np.asarray(out_arrs[i]).reshape(n_cores, *out_avals[i].shape)[c]
             for i, name in enumerate(out_names)}
            for c in range(n_cores)
        ]
        state["zeros"] = _stage_zeros()     # async, off the timed path
        return results

    return run


def _stage_inputs(points, cov_factor, colors, opacity, extrinsic, fx, fy):
    """Project gaussians on host (f64), depth-sort, cull globally, shard the
    sorted list across cores, pack each shard into one [128, CSH] tensor."""
    N = points.shape[0]
    pts = np.asarray(points, np.float32)
    ex = np.asarray(extrinsic, np.float32)

    # depth order as the reference computes it (f32 matmul; verified
    # bit-identical to the jax cpu matmul the reference uses)
    ph = np.concatenate([pts, np.ones((N, 1), np.float32)], axis=1)
    z32 = (ph @ ex)[:, 2]
    order = np.argsort(z32, kind="stable")

    pc = ph.astype(np.float64) @ ex.astype(np.float64)
    x, y, z = pc[:, 0], pc[:, 1], pc[:, 2]
    zs = np.where(z == 0.0, 1e-30, z)
    cf = np.asarray(cov_factor, np.float64)
    cov3d = 0.05 * np.matmul(cf, cf.transpose(0, 2, 1)) + 1e-4 * np.eye(3)
    Rm = ex[:3, :3].astype(np.float64).T
    J = np.zeros((N, 2, 3))
    J[:, 0, 0] = fx / zs
    J[:, 0, 2] = fx * x / zs**2
    J[:, 1, 1] = fy / zs
    J[:, 1, 2] = fy * y / zs**2
    T = np.matmul(J, Rm)
    cov2d = np.matmul(np.matmul(T, cov3d), T.transpose(0, 2, 1))
    a, b_, c = cov2d[:, 0, 0], cov2d[:, 0, 1], cov2d[:, 1, 1]
    det = a * c - b_ * b_
    inv_det = 1.0 / np.maximum(det, 1e-12)
    m05ia = -0.5 * c * inv_det
    m05ic = -0.5 * a * inv_det
    mib = b_ * inv_det
    mid = 0.5 * (a + c)
    lam = mid + np.sqrt(np.maximum(mid * mid - det, 0.1))
    rad = np.ceil(3.0 * np.sqrt(np.maximum(lam, 0.0)))
    rad = np.nan_to_num(rad, nan=1e9, posinf=1e9)
    tfx = W / (2.0 * fx)
    tfy = H / (2.0 * fy)
    pxp = fx * np.clip(x / zs, -1.3 * tfx, 1.3 * tfx) + 0.5 * W
    pyp = fy * np.clip(y / zs, -1.3 * tfy, 1.3 * tfy) + 0.5 * H
    in_view = (z > ZNEAR) & (det > 0)
    opac = np.asarray(opacity, np.float64)
    lsigm = np.where(in_view, -np.logaddexp(0.0, -opac), -BIGNEG)

    m05ia = np.where(in_view, m05ia, 0.0)
    m05ic = np.where(in_view, m05ic, 0.0)
    mib = np.where(in_view, mib, 0.0)
    pxp = np.where(in_view, pxp, 0.0)
    pyp = np.where(in_view, pyp, 0.0)
    rad = np.where(in_view, rad, -1.0)

    # global cull: drop gaussians invisible to the whole image
    M = 2.0
    kill = (~in_view) | (pxp + rad < -M) | (pxp - rad > W - 1 + M) \
        | (pyp + rad < -M) | (pyp - rad > H - 1 + M)
    keep = order[~kill[order]]
    n = len(keep)
    nb = NCORES * max(1, int(np.ceil(n / (128.0 * NCORES))))
    nbs = nb // NCORES
    CSH = 10 * nbs + ROWS

    cols = np.asarray(colors, np.float32)
    planes = [(pxp, 0.0), (pyp, 0.0), (m05ia, 0.0), (m05ic, 0.0),
              (mib, 0.0), (rad, -1.0), (lsigm, -BIGNEG)]

    # pack the full sorted list block-major, then split into per-core shards
    full = np.zeros((128, 10 * nb), np.float32)
    for p, (arr, padval) in enumerate(planes):
        col = np.full(nb * 128, padval, np.float32)
        col[:n] = arr[keep]
        # plane p of shard s occupies [10*nbs*s + p*nbs, ... + nbs)
        bm = col.reshape(nb, 128).T        # [128, nb] block-major
        for s in range(NCORES):
            full[:, 10 * nbs * s + p * nbs: 10 * nbs * s + (p + 1) * nbs] = \
                bm[:, s * nbs:(s + 1) * nbs]
    padded = np.zeros((nb * 128, 3), np.float32)
    padded[:n] = cols[keep]
    for b in range(nb):
        s, k = divmod(b, nbs)
        full[:, 10 * nbs * s + 7 * nbs + 3 * k: 10 * nbs * s + 7 * nbs + 3 * k + 3] = \
            padded[b * 128:(b + 1) * 128]

    in_maps = []
    for cidx in range(NCORES):
        pkarr = np.zeros((128, CSH), np.float32)
        pkarr[:, :10 * nbs] = full[:, 10 * nbs * cidx:10 * nbs * (cidx + 1)]
        pkarr[:, 10 * nbs:] = np.arange(cidx * ROWS, (cidx + 1) * ROWS,
                                        dtype=np.float32)
        in_maps.append({"pk": pkarr})

    sig = 1.0 / (1.0 + np.exp(-float(np.asarray(opacity, np.float64).max())))
    use_clamp = bool(sig > 0.985)
    return in_maps, nb, use_clamp


def kernel(points, cov_factor, colors, opacity, extrinsic, focal_x, focal_y,
           width, height):
    fx, fy = float(focal_x), float(focal_y)
    assert int(width) == W and int(height) == H

    in_maps, nb, use_clamp = _stage_inputs(points, cov_factor, colors, opacity,
                                           extrinsic, fx, fy)
    key = (nb, use_clamp)
    if key not in _program_cache:
        nc = _build_program(*key)
        _program_cache[key] = (nc, _make_runner(nc))
    nc, run = _program_cache[key]

    results = run(in_maps)

    out = np.zeros((H, W, 3), np.float32)
    for cidx in range(NCORES):
        band = results[cidx]["img"].reshape(3, ROWS, W)
        out[cidx * ROWS:(cidx + 1) * ROWS] = band.transpose(1, 2, 0)
    return out


# revision 5
# speedup vs baseline: 1.1858x; 1.1858x over previous
"""Trainium2 Bass kernel for GaussianScene2 (3D gaussian splatting renderer).

Data-parallel over image row-bands: each of 8 cores composites one 16-row
band (2048 px). Per-gaussian projection (EWA 2D covariance inverse, pixel
means, radius, log-sigmoid opacity) is precomputed on host in f64, packed
into 10 block-major planes, and sharded across the cores: each core ships
only 1/8th of the depth-sorted list (~21KB) and an on-device AllGather over
NeuronLink reconstructs the full list, cutting host->device traffic over
the PJRT tunnel ~8x. On device, per-block [128 gaussians x 2048 px] tiles
evaluate the 2D gaussians; alpha converts to log-transmittance, and the
front-to-back compositing cumsum runs as triangular matmuls on the PE
engine with the across-block carry accumulated in PSUM; colors accumulate
via a second matmul into a [3, 2048] PSUM image. Pixel ramps and triangular
masks are iota-generated on device. The PJRT executable wrapper is built
once and cached (run_bass_kernel_spmd would retrace it every call, ~200ms),
and the donated output zero-buffers are pre-staged on device between calls
so the timed path pays only input upload + dispatch + execute + fetch.
"""

import sys

sys.path.insert(0, "/opt/trn_rl_repo")

import numpy as np

H = 128
W = 128
NCORES = 8
ROWS = H // NCORES          # rows per core
NPIX = ROWS * W             # pixels per core
CHUNK = 512                 # psum bank free size (fp32)
NCH = NPIX // CHUNK
ZNEAR = 0.2
MIN_T = 0.01
BIGNEG = 1.0e30

_program_cache = {}


def _build_program(nb, use_clamp):
    """nb = TOTAL gaussian blocks (multiple of NCORES); each core ships nb/8."""
    from contextlib import ExitStack

    import concourse.bacc as bacc
    import concourse.tile as tile
    from concourse import mybir

    F32 = mybir.dt.float32
    AF = mybir.ActivationFunctionType
    ALU = mybir.AluOpType
    LNMINT = float(np.log(np.float32(MIN_T)))

    assert nb % NCORES == 0
    nbs = nb // NCORES              # blocks per shard
    CSH = 10 * nbs + ROWS           # per-core input cols
    CG = 10 * nbs                   # gathered cols per shard

    nc = bacc.Bacc("TRN2", target_bir_lowering=False, debug=False)

    pk_d = nc.dram_tensor("pk", [128, CSH], F32, kind="ExternalInput")
    img_d = nc.dram_tensor("img", [3, NPIX], F32, kind="ExternalOutput")
    gin = nc.dram_tensor("gin", [128, CG], F32)
    gout = nc.dram_tensor("gout", [NCORES, 128, CG], F32, addr_space="Shared")

    with tile.TileContext(nc) as tc, ExitStack() as ctx:
        P = ctx.enter_context(tc.tile_pool(name="pre", bufs=1))
        WK = ctx.enter_context(tc.tile_pool(name="work", bufs=2))
        PS = ctx.enter_context(tc.tile_pool(name="psum", bufs=1, space="PSUM"))

        def pt(shape, tag):
            return P.tile(shape, F32, tag=tag, name=tag)

        # ---- shard in, AllGather, unpack to SBUF ----
        nc.sync.dma_start(gin[:], pk_d[:, :CG])
        nc.gpsimd.collective_compute(
            "AllGather", ALU.bypass, replica_groups=[list(range(NCORES))],
            ins=[gin[:]], outs=[gout[:]])

        pl = pt([128, 7, nb], "pl")          # 7 planes x all blocks
        colT = pt([128, 3 * nb], "colT")     # interleaved colors per block
        for s in range(NCORES):
            src = gout[s]                    # [128, CG]
            nc.sync.dma_start(
                pl[:, :, s * nbs:(s + 1) * nbs],
                src[:, :7 * nbs].rearrange("p (t n) -> p t n", t=7))
            nc.sync.dma_start(
                colT[:, 3 * nbs * s:3 * nbs * (s + 1)],
                src[:, 7 * nbs:])
        rowg_t = pt([128, ROWS], "rowg")
        nc.sync.dma_start(rowg_t[:], pk_d[:, CG:])

        px = pl[:, 0, :]
        py = pl[:, 1, :]
        m05ia = pl[:, 2, :]
        m05ic = pl[:, 3, :]
        mib = pl[:, 4, :]
        rad = pl[:, 5, :]
        lsigm = pl[:, 6, :]
        rowg = rowg_t[:]

        ts_ = nc.vector.tensor_scalar
        ttv = nc.vector.tensor_tensor
        ttp = nc.gpsimd.tensor_tensor
        act = nc.scalar.activation

        # ---- on-device constants: pixel-x ramp, row index, triangular masks
        gx = pt([128, 128], "gx")
        nc.gpsimd.iota(gx[:], [[1, 128]], channel_multiplier=0,
                       allow_small_or_imprecise_dtypes=True)
        rix = pt([128, 128], "rix")
        nc.gpsimd.iota(rix[:], [[0, 128]], channel_multiplier=1,
                       allow_small_or_imprecise_dtypes=True)
        tris = pt([128, 128], "tris")
        ttv(out=tris[:], in0=rix[:], in1=gx[:], op=ALU.is_le)
        lows = pt([128, 128], "lows")
        ttv(out=lows[:], in0=rix[:], in1=gx[:], op=ALU.is_gt)

        # ---- per-block pixel-x precompute: qxm[g, b, w], bxw[g, b, w] ----
        qxm = pt([128, nb, 128], "qxm")
        bxw = pt([128, nb, 128], "bxw")
        dxw = WK.tile([128, nb, 128], F32, tag="dxw", name="dxw")
        tmpx = WK.tile([128, nb, 128], F32, tag="tmpx", name="tmpx")
        gx_b = gx[:].unsqueeze(1).broadcast_to([128, nb, 128])
        px_b = px.unsqueeze(2).broadcast_to([128, nb, 128])
        rad_b = rad.unsqueeze(2).broadcast_to([128, nb, 128])
        ttp(out=dxw[:], in0=gx_b, in1=px_b, op=ALU.subtract)
        act(out=tmpx[:], in_=dxw[:], func=AF.Abs)
        ttv(out=tmpx[:], in0=tmpx[:], in1=rad_b, op=ALU.is_le)
        ts_(out=tmpx[:], in0=tmpx[:], scalar1=BIGNEG, scalar2=BIGNEG, op0=ALU.mult, op1=ALU.subtract)
        m05ia_b = m05ia.unsqueeze(2).broadcast_to([128, nb, 128])
        ttp(out=qxm[:], in0=dxw[:], in1=dxw[:], op=ALU.mult)
        ttp(out=qxm[:], in0=qxm[:], in1=m05ia_b, op=ALU.mult)
        ttp(out=qxm[:], in0=qxm[:], in1=tmpx[:], op=ALU.add)
        mib_b = mib.unsqueeze(2).broadcast_to([128, nb, 128])
        ttp(out=bxw[:], in0=dxw[:], in1=mib_b, op=ALU.mult)

        # ---- per-block row precompute: dyr[g, b, r], sylm[g, b, r] ----
        dyr = pt([128, nb, ROWS], "dyr")
        sylm = pt([128, nb, ROWS], "sylm")
        tmpy = WK.tile([128, nb, ROWS], F32, tag="tmpy", name="tmpy")
        rowg_b = rowg.unsqueeze(1).broadcast_to([128, nb, ROWS])
        py_b = py.unsqueeze(2).broadcast_to([128, nb, ROWS])
        radr_b = rad.unsqueeze(2).broadcast_to([128, nb, ROWS])
        m05ic_b = m05ic.unsqueeze(2).broadcast_to([128, nb, ROWS])
        ttp(out=dyr[:], in0=rowg_b, in1=py_b, op=ALU.subtract)
        act(out=tmpy[:], in_=dyr[:], func=AF.Abs)
        ttv(out=tmpy[:], in0=tmpy[:], in1=radr_b, op=ALU.is_le)
        ts_(out=tmpy[:], in0=tmpy[:], scalar1=BIGNEG, scalar2=BIGNEG, op0=ALU.mult, op1=ALU.subtract)
        ttp(out=sylm[:], in0=dyr[:], in1=dyr[:], op=ALU.mult)
        ttp(out=sylm[:], in0=sylm[:], in1=m05ic_b, op=ALU.mult)
        ttp(out=sylm[:], in0=sylm[:], in1=tmpy[:], op=ALU.add)

        # ---- main compositing loop over gaussian blocks ----
        psS = PS.tile([128, NPIX], F32, tag="psS", name="psS")
        psI = PS.tile([3, NPIX], F32, tag="psI", name="psI")

        for b in range(nb):
            power = WK.tile([128, ROWS, 128], F32, tag="power", name="power")
            bx_b = bxw[:, b, :].unsqueeze(1).broadcast_to([128, ROWS, 128])
            dy_b = dyr[:, b, :].unsqueeze(2).broadcast_to([128, ROWS, 128])
            qx_b = qxm[:, b, :].unsqueeze(1).broadcast_to([128, ROWS, 128])
            sy_b = sylm[:, b, :].unsqueeze(2).broadcast_to([128, ROWS, 128])
            ttp(out=power[:], in0=bx_b, in1=dy_b, op=ALU.mult)
            ttp(out=power[:], in0=power[:], in1=qx_b, op=ALU.add)
            ttv(out=power[:], in0=power[:], in1=sy_b, op=ALU.add)
            pw = power[:].rearrange("g r w -> g (r w)")
            ls_b = lsigm[:, b:b + 1]
            ts_(out=pw, in0=pw, scalar1=ls_b, scalar2=ls_b, op0=ALU.add, op1=ALU.min)
            alpha = WK.tile([128, NPIX], F32, tag="alpha", name="alpha")
            act(out=alpha[:], in_=pw, func=AF.Exp)
            if use_clamp:
                ts_(out=alpha[:], in0=alpha[:], scalar1=0.99, scalar2=None, op0=ALU.min)
            lt = WK.tile([128, NPIX], F32, tag="lt", name="lt")
            act(out=lt[:], in_=alpha[:], func=AF.Ln, scale=-1.0, bias=1.0)

            for k in range(NCH):
                sl = slice(k * CHUNK, (k + 1) * CHUNK)
                nc.tensor.matmul(out=psS[:, sl], lhsT=tris[:],
                                 rhs=lt[:, sl],
                                 start=(b == 0), stop=True,
                                 skip_group_check=(b != 0))

            sprev = WK.tile([128, NPIX], F32, tag="power", name="sprev")
            maskt = WK.tile([128, NPIX], F32, tag="alpha", name="alpha")
            for k in range(NCH):
                sl = slice(k * CHUNK, (k + 1) * CHUNK)
                ttv(out=sprev[:, sl], in0=psS[:, sl], in1=lt[:, sl], op=ALU.subtract)
                ts_(out=maskt[:, sl], in0=psS[:, sl], scalar1=LNMINT, scalar2=None,
                    op0=ALU.is_ge)
            tprev = WK.tile([128, NPIX], F32, tag="lt", name="lt")
            act(out=tprev[:], in_=sprev[:], func=AF.Exp)
            contrib = WK.tile([128, NPIX], F32, tag="contrib", name="contrib")
            nc.gpsimd.tensor_tensor(out=contrib[:], in0=tprev[:], in1=alpha[:], op=ALU.mult)
            half = NPIX // 2
            ttp(out=contrib[:, :half], in0=contrib[:, :half],
                in1=maskt[:, :half], op=ALU.mult)
            nc.gpsimd.tensor_tensor(out=contrib[:, half:], in0=contrib[:, half:],
                                    in1=maskt[:, half:], op=ALU.mult)

            for k in range(NCH):
                sl = slice(k * CHUNK, (k + 1) * CHUNK)
                nc.tensor.matmul(out=psI[:, sl],
                                 lhsT=colT[:, 3 * b:3 * b + 3],
                                 rhs=contrib[:, sl],
                                 start=(b == 0), stop=True,
                                 skip_group_check=(b != 0))

            if b != nb - 1:
                for k in range(NCH):
                    sl = slice(k * CHUNK, (k + 1) * CHUNK)
                    nc.tensor.matmul(out=psS[:, sl], lhsT=lows[:],
                                     rhs=lt[:, sl],
                                     start=False, stop=True, skip_group_check=True)

        imgsb = P.tile([3, NPIX], F32, tag="imgsb", name="imgsb")
        for k in range(NCH):
            sl = slice(k * CHUNK, (k + 1) * CHUNK)
            nc.vector.tensor_copy(out=imgsb[:, sl], in_=psI[:, sl])
        nc.sync.dma_start(img_d[:], imgsb[:])

    nc.compile()
    return nc


def _make_runner(nc, n_cores=NCORES):
    import jax
    from jax.sharding import Mesh, PartitionSpec
    from jax.experimental.shard_map import shard_map

    from concourse import mybir
    from concourse.bass2jax import (_bass_exec_p, install_neuronx_cc_hook,
                                    partition_id_tensor)

    install_neuronx_cc_hook()
    pn = nc.partition_id_tensor.name if nc.partition_id_tensor else None
    in_names, out_names, out_avals, zero_outs = [], [], [], []
    for alloc in nc.m.functions[0].allocations:
        if not isinstance(alloc, mybir.MemoryLocationSet):
            continue
        name = alloc.memorylocations[0].name
        if alloc.kind == "ExternalInput":
            if name != pn:
                in_names.append(name)
        elif alloc.kind == "ExternalOutput":
            shape = tuple(alloc.tensor_shape)
            dtype = mybir.dt.np(alloc.dtype)
            out_names.append(name)
            out_avals.append(jax.core.ShapedArray(shape, dtype))
            zero_outs.append(np.zeros(shape, dtype))
    n_params = len(in_names)
    n_outs = len(out_avals)
    in_all = in_names + out_names + ([pn] if pn else [])
    donate = tuple(range(n_params, n_params + n_outs))

    def _body(*args):
        ops = list(args)
        if pn is not None:
            ops.append(partition_id_tensor())
        return tuple(_bass_exec_p.bind(
            *ops, out_avals=tuple(out_avals), in_names=tuple(in_all),
            out_names=tuple(out_names), lowering_input_output_aliases=(),
            sim_require_finite=True, sim_require_nnan=True, nc=nc))

    mesh = Mesh(np.asarray(jax.devices()[:n_cores]), ("core",))
    fn = jax.jit(
        shard_map(_body, mesh=mesh,
                  in_specs=(PartitionSpec("core"),) * (n_params + n_outs),
                  out_specs=(PartitionSpec("core"),) * len(out_names),
                  check_rep=False),
        donate_argnums=donate, keep_unused=True)

    # The donated output buffers are a PJRT output-binding artifact: the NEFF
    # writes every element of img (verified: a garbage-filled buffer yields a
    # bit-identical image), so their contents never matter. Recycle the
    # previous call's output arrays as the next call's donated buffers --
    # they are already on device, so no zero-buffer ever rides the tunnel.
    from jax.sharding import NamedSharding
    zsharding = NamedSharding(mesh, PartitionSpec("core"))
    state = {"bufs": None}

    def _fresh_bufs():
        return [
            jax.device_put(
                np.zeros((n_cores * z.shape[0], *z.shape[1:]), z.dtype),
                zsharding)
            for z in zero_outs
        ]

    def run(in_maps):
        concat_in = [
            np.concatenate([np.asarray(m[name]) for m in in_maps], axis=0)
            for name in in_names
        ]
        bufs = state["bufs"] if state["bufs"] is not None else _fresh_bufs()
        state["bufs"] = None                # consumed by donation below
        out_arrs = fn(*concat_in, *bufs)
        results = [
            {name: np.asarray(out_arrs[i]).reshape(n_cores, *out_avals[i].shape)[c]
             for i, name in enumerate(out_names)}
            for c in range(n_cores)
        ]
        state["bufs"] = list(out_arrs)      # recycle for the next call
        return results

    return run


def _stage_inputs(points, cov_factor, colors, opacity, extrinsic, fx, fy):
    """Project gaussians on host (f64), depth-sort, cull globally, shard the
    sorted list across cores, pack each shard into one [128, CSH] tensor."""
    N = points.shape[0]
    pts = np.asarray(points, np.float32)
    ex = np.asarray(extrinsic, np.float32)

    # depth order as the reference computes it (f32 matmul; verified
    # bit-identical to the jax cpu matmul the reference uses)
    ph = np.concatenate([pts, np.ones((N, 1), np.float32)], axis=1)
    z32 = (ph @ ex)[:, 2]
    order = np.argsort(z32, kind="stable")

    pc = ph.astype(np.float64) @ ex.astype(np.float64)
    x, y, z = pc[:, 0], pc[:, 1], pc[:, 2]
    zs = np.where(z == 0.0, 1e-30, z)
    cf = np.asarray(cov_factor, np.float64)
    cov3d = 0.05 * np.matmul(cf, cf.transpose(0, 2, 1)) + 1e-4 * np.eye(3)
    Rm = ex[:3, :3].astype(np.float64).T
    J = np.zeros((N, 2, 3))
    J[:, 0, 0] = fx / zs
    J[:, 0, 2] = fx * x / zs**2
    J[:, 1, 1] = fy / zs
    J[:, 1, 2] = fy * y / zs**2
    T = np.matmul(J, Rm)
    cov2d = np.matmul(np.matmul(T, cov3d), T.transpose(0, 2, 1))
    a, b_, c = cov2d[:, 0, 0], cov2d[:, 0, 1], cov2d[:, 1, 1]
    det = a * c - b_ * b_
    inv_det = 1.0 / np.maximum(det, 1e-12)
    m05ia = -0.5 * c * inv_det
    m05ic = -0.5 * a * inv_det
    mib = b_ * inv_det
    mid = 0.5 * (a + c)
    lam = mid + np.sqrt(np.maximum(mid * mid - det, 0.1))
    rad = np.ceil(3.0 * np.sqrt(np.maximum(lam, 0.0)))
    rad = np.nan_to_num(rad, nan=1e9, posinf=1e9)
    tfx = W / (2.0 * fx)
    tfy = H / (2.0 * fy)
    pxp = fx * np.clip(x / zs, -1.3 * tfx, 1.3 * tfx) + 0.5 * W
    pyp = fy * np.clip(y / zs, -1.3 * tfy, 1.3 * tfy) + 0.5 * H
    in_view = (z > ZNEAR) & (det > 0)
    opac = np.asarray(opacity, np.float64)
    lsigm = np.where(in_view, -np.logaddexp(0.0, -opac), -BIGNEG)

    m05ia = np.where(in_view, m05ia, 0.0)
    m05ic = np.where(in_view, m05ic, 0.0)
    mib = np.where(in_view, mib, 0.0)
    pxp = np.where(in_view, pxp, 0.0)
    pyp = np.where(in_view, pyp, 0.0)
    rad = np.where(in_view, rad, -1.0)

    # global cull: drop gaussians invisible to the whole image
    M = 2.0
    kill = (~in_view) | (pxp + rad < -M) | (pxp - rad > W - 1 + M) \
        | (pyp + rad < -M) | (pyp - rad > H - 1 + M)
    keep = order[~kill[order]]
    n = len(keep)
    nb = NCORES * max(1, int(np.ceil(n / (128.0 * NCORES))))
    nbs = nb // NCORES
    CSH = 10 * nbs + ROWS

    cols = np.asarray(colors, np.float32)
    planes = [(pxp, 0.0), (pyp, 0.0), (m05ia, 0.0), (m05ic, 0.0),
              (mib, 0.0), (rad, -1.0), (lsigm, -BIGNEG)]

    # pack the full sorted list block-major, then split into per-core shards
    full = np.zeros((128, 10 * nb), np.float32)
    for p, (arr, padval) in enumerate(planes):
        col = np.full(nb * 128, padval, np.float32)
        col[:n] = arr[keep]
        # plane p of shard s occupies [10*nbs*s + p*nbs, ... + nbs)
        bm = col.reshape(nb, 128).T        # [128, nb] block-major
        for s in range(NCORES):
            full[:, 10 * nbs * s + p * nbs: 10 * nbs * s + (p + 1) * nbs] = \
                bm[:, s * nbs:(s + 1) * nbs]
    padded = np.zeros((nb * 128, 3), np.float32)
    padded[:n] = cols[keep]
    for b in range(nb):
        s, k = divmod(b, nbs)
        full[:, 10 * nbs * s + 7 * nbs + 3 * k: 10 * nbs * s + 7 * nbs + 3 * k + 3] = \
            padded[b * 128:(b + 1) * 128]

    in_maps = []
    for cidx in range(NCORES):
        pkarr = np.zeros((128, CSH), np.float32)
        pkarr[:, :10 * nbs] = full[:, 10 * nbs * cidx:10 * nbs * (cidx + 1)]
        pkarr[:, 10 * nbs:] = np.arange(cidx * ROWS, (cidx + 1) * ROWS,
                                        dtype=np.float32)
        in_maps.append({"pk": pkarr})

    sig = 1.0 / (1.0 + np.exp(-float(np.asarray(opacity, np.float64).max())))
    use_clamp = bool(sig > 0.985)
    return in_maps, nb, use_clamp


def kernel(points, cov_factor, colors, opacity, extrinsic, focal_x, focal_y,
           width, height):
    fx, fy = float(focal_x), float(focal_y)
    assert int(width) == W and int(height) == H

    in_maps, nb, use_clamp = _stage_inputs(points, cov_factor, colors, opacity,
                                           extrinsic, fx, fy)
    key = (nb, use_clamp)
    if key not in _program_cache:
        nc = _build_program(*key)
        _program_cache[key] = (nc, _make_runner(nc))
    nc, run = _program_cache[key]

    results = run(in_maps)

    out = np.zeros((H, W, 3), np.float32)
    for cidx in range(NCORES):
        band = results[cidx]["img"].reshape(3, ROWS, W)
        out[cidx * ROWS:(cidx + 1) * ROWS] = band.transpose(1, 2, 0)
    return out


# revision 6
# speedup vs baseline: 1.2250x; 1.0330x over previous
"""Trainium2 Bass kernel for GaussianScene2 (3D gaussian splatting renderer).

Data-parallel over image row-bands: each of 8 cores composites one 16-row
band (2048 px). Per-gaussian projection (EWA 2D covariance inverse, pixel
means, radius, log-sigmoid opacity) is precomputed on host in f64, packed
into 10 block-major planes, and sharded across the cores: each core ships
only 1/8th of the depth-sorted list (~21KB) and an on-device AllGather over
NeuronLink reconstructs the full list, cutting host->device traffic over
the PJRT tunnel ~8x. On device, per-block [128 gaussians x 2048 px] tiles
evaluate the 2D gaussians; alpha converts to log-transmittance, and the
front-to-back compositing cumsum runs as triangular matmuls on the PE
engine with the across-block carry accumulated in PSUM; colors accumulate
via a second matmul into a [3, 2048] PSUM image. Pixel ramps and triangular
masks are iota-generated on device. The PJRT executable wrapper is built
once and cached (run_bass_kernel_spmd would retrace it every call, ~200ms),
and the donated output zero-buffers are pre-staged on device between calls
so the timed path pays only input upload + dispatch + execute + fetch.
"""

import sys

sys.path.insert(0, "/opt/trn_rl_repo")

import numpy as np

H = 128
W = 128
NCORES = 8
ROWS = H // NCORES          # rows per core
NPIX = ROWS * W             # pixels per core
CHUNK = 512                 # psum bank free size (fp32)
NCH = NPIX // CHUNK
ZNEAR = 0.2
MIN_T = 0.01
BIGNEG = 1.0e30

_program_cache = {}


def _build_program(nb, use_clamp):
    """nb = TOTAL gaussian blocks (multiple of NCORES); each core ships nb/8."""
    from contextlib import ExitStack

    import concourse.bacc as bacc
    import concourse.tile as tile
    from concourse import mybir

    F32 = mybir.dt.float32
    AF = mybir.ActivationFunctionType
    ALU = mybir.AluOpType
    LNMINT = float(np.log(np.float32(MIN_T)))

    assert nb % NCORES == 0
    nbs = nb // NCORES              # blocks per shard
    CSH = 10 * nbs + ROWS           # per-core input cols
    CG = 10 * nbs                   # gathered cols per shard

    nc = bacc.Bacc("TRN2", target_bir_lowering=False, debug=False)

    pk_d = nc.dram_tensor("pk", [128, CSH], F32, kind="ExternalInput")
    img_d = nc.dram_tensor("img", [3, NPIX], F32, kind="ExternalOutput")
    gin = nc.dram_tensor("gin", [128, CG], F32)
    gout = nc.dram_tensor("gout", [NCORES, 128, CG], F32, addr_space="Shared")

    with tile.TileContext(nc) as tc, ExitStack() as ctx:
        P = ctx.enter_context(tc.tile_pool(name="pre", bufs=1))
        WK = ctx.enter_context(tc.tile_pool(name="work", bufs=2))
        PS = ctx.enter_context(tc.tile_pool(name="psum", bufs=1, space="PSUM"))

        def pt(shape, tag):
            return P.tile(shape, F32, tag=tag, name=tag)

        # ---- shard in, AllGather, unpack to SBUF ----
        nc.sync.dma_start(gin[:], pk_d[:, :CG])
        nc.gpsimd.collective_compute(
            "AllGather", ALU.bypass, replica_groups=[list(range(NCORES))],
            ins=[gin[:]], outs=[gout[:]])

        pl = pt([128, 7, nb], "pl")          # 7 planes x all blocks
        colT = pt([128, 3 * nb], "colT")     # interleaved colors per block
        for s in range(NCORES):
            src = gout[s]                    # [128, CG]
            nc.sync.dma_start(
                pl[:, :, s * nbs:(s + 1) * nbs],
                src[:, :7 * nbs].rearrange("p (t n) -> p t n", t=7))
            nc.sync.dma_start(
                colT[:, 3 * nbs * s:3 * nbs * (s + 1)],
                src[:, 7 * nbs:])
        rowg_t = pt([128, ROWS], "rowg")
        nc.sync.dma_start(rowg_t[:], pk_d[:, CG:])

        px = pl[:, 0, :]
        py = pl[:, 1, :]
        m05ia = pl[:, 2, :]
        m05ic = pl[:, 3, :]
        mib = pl[:, 4, :]
        rad = pl[:, 5, :]
        lsigm = pl[:, 6, :]
        rowg = rowg_t[:]

        ts_ = nc.vector.tensor_scalar
        ttv = nc.vector.tensor_tensor
        ttp = nc.gpsimd.tensor_tensor
        act = nc.scalar.activation

        # ---- on-device constants: pixel-x ramp, row index, triangular masks
        gx = pt([128, 128], "gx")
        nc.gpsimd.iota(gx[:], [[1, 128]], channel_multiplier=0,
                       allow_small_or_imprecise_dtypes=True)
        rix = pt([128, 128], "rix")
        nc.gpsimd.iota(rix[:], [[0, 128]], channel_multiplier=1,
                       allow_small_or_imprecise_dtypes=True)
        tris = pt([128, 128], "tris")
        ttv(out=tris[:], in0=rix[:], in1=gx[:], op=ALU.is_le)
        lows = pt([128, 128], "lows")
        ttv(out=lows[:], in0=rix[:], in1=gx[:], op=ALU.is_gt)

        # ---- per-block pixel-x precompute: qxm[g, b, w], bxw[g, b, w] ----
        qxm = pt([128, nb, 128], "qxm")
        bxw = pt([128, nb, 128], "bxw")
        dxw = WK.tile([128, nb, 128], F32, tag="dxw", name="dxw")
        tmpx = WK.tile([128, nb, 128], F32, tag="tmpx", name="tmpx")
        gx_b = gx[:].unsqueeze(1).broadcast_to([128, nb, 128])
        px_b = px.unsqueeze(2).broadcast_to([128, nb, 128])
        rad_b = rad.unsqueeze(2).broadcast_to([128, nb, 128])
        ttp(out=dxw[:], in0=gx_b, in1=px_b, op=ALU.subtract)
        act(out=tmpx[:], in_=dxw[:], func=AF.Abs)
        ttv(out=tmpx[:], in0=tmpx[:], in1=rad_b, op=ALU.is_le)
        ts_(out=tmpx[:], in0=tmpx[:], scalar1=BIGNEG, scalar2=BIGNEG, op0=ALU.mult, op1=ALU.subtract)
        m05ia_b = m05ia.unsqueeze(2).broadcast_to([128, nb, 128])
        ttp(out=qxm[:], in0=dxw[:], in1=dxw[:], op=ALU.mult)
        ttp(out=qxm[:], in0=qxm[:], in1=m05ia_b, op=ALU.mult)
        ttp(out=qxm[:], in0=qxm[:], in1=tmpx[:], op=ALU.add)
        mib_b = mib.unsqueeze(2).broadcast_to([128, nb, 128])
        ttp(out=bxw[:], in0=dxw[:], in1=mib_b, op=ALU.mult)

        # ---- per-block row precompute: dyr[g, b, r], sylm[g, b, r] ----
        dyr = pt([128, nb, ROWS], "dyr")
        sylm = pt([128, nb, ROWS], "sylm")
        tmpy = WK.tile([128, nb, ROWS], F32, tag="tmpy", name="tmpy")
        rowg_b = rowg.unsqueeze(1).broadcast_to([128, nb, ROWS])
        py_b = py.unsqueeze(2).broadcast_to([128, nb, ROWS])
        radr_b = rad.unsqueeze(2).broadcast_to([128, nb, ROWS])
        m05ic_b = m05ic.unsqueeze(2).broadcast_to([128, nb, ROWS])
        ttp(out=dyr[:], in0=rowg_b, in1=py_b, op=ALU.subtract)
        act(out=tmpy[:], in_=dyr[:], func=AF.Abs)
        ttv(out=tmpy[:], in0=tmpy[:], in1=radr_b, op=ALU.is_le)
        ts_(out=tmpy[:], in0=tmpy[:], scalar1=BIGNEG, scalar2=BIGNEG, op0=ALU.mult, op1=ALU.subtract)
        ttp(out=sylm[:], in0=dyr[:], in1=dyr[:], op=ALU.mult)
        ttp(out=sylm[:], in0=sylm[:], in1=m05ic_b, op=ALU.mult)
        ttp(out=sylm[:], in0=sylm[:], in1=tmpy[:], op=ALU.add)

        # ---- main compositing loop over gaussian blocks ----
        psS = PS.tile([128, NPIX], F32, tag="psS", name="psS")
        psI = PS.tile([3, NPIX], F32, tag="psI", name="psI")

        for b in range(nb):
            power = WK.tile([128, ROWS, 128], F32, tag="power", name="power")
            bx_b = bxw[:, b, :].unsqueeze(1).broadcast_to([128, ROWS, 128])
            dy_b = dyr[:, b, :].unsqueeze(2).broadcast_to([128, ROWS, 128])
            qx_b = qxm[:, b, :].unsqueeze(1).broadcast_to([128, ROWS, 128])
            sy_b = sylm[:, b, :].unsqueeze(2).broadcast_to([128, ROWS, 128])
            ttp(out=power[:], in0=bx_b, in1=dy_b, op=ALU.mult)
            ttp(out=power[:], in0=power[:], in1=qx_b, op=ALU.add)
            ttv(out=power[:], in0=power[:], in1=sy_b, op=ALU.add)
            pw = power[:].rearrange("g r w -> g (r w)")
            ls_b = lsigm[:, b:b + 1]
            ts_(out=pw, in0=pw, scalar1=ls_b, scalar2=ls_b, op0=ALU.add, op1=ALU.min)
            alpha = WK.tile([128, NPIX], F32, tag="alpha", name="alpha")
            act(out=alpha[:], in_=pw, func=AF.Exp)
            if use_clamp:
                ts_(out=alpha[:], in0=alpha[:], scalar1=0.99, scalar2=None, op0=ALU.min)
            lt = WK.tile([128, NPIX], F32, tag="lt", name="lt")
            act(out=lt[:], in_=alpha[:], func=AF.Ln, scale=-1.0, bias=1.0)

            for k in range(NCH):
                sl = slice(k * CHUNK, (k + 1) * CHUNK)
                nc.tensor.matmul(out=psS[:, sl], lhsT=tris[:],
                                 rhs=lt[:, sl],
                                 start=(b == 0), stop=True,
                                 skip_group_check=(b != 0))

            sprev = WK.tile([128, NPIX], F32, tag="power", name="sprev")
            maskt = WK.tile([128, NPIX], F32, tag="alpha", name="alpha")
            for k in range(NCH):
                sl = slice(k * CHUNK, (k + 1) * CHUNK)
                ttv(out=sprev[:, sl], in0=psS[:, sl], in1=lt[:, sl], op=ALU.subtract)
                ts_(out=maskt[:, sl], in0=psS[:, sl], scalar1=LNMINT, scalar2=None,
                    op0=ALU.is_ge)
            tprev = WK.tile([128, NPIX], F32, tag="lt", name="lt")
            act(out=tprev[:], in_=sprev[:], func=AF.Exp)
            contrib = WK.tile([128, NPIX], F32, tag="contrib", name="contrib")
            nc.gpsimd.tensor_tensor(out=contrib[:], in0=tprev[:], in1=alpha[:], op=ALU.mult)
            half = NPIX // 2
            ttp(out=contrib[:, :half], in0=contrib[:, :half],
                in1=maskt[:, :half], op=ALU.mult)
            nc.gpsimd.tensor_tensor(out=contrib[:, half:], in0=contrib[:, half:],
                                    in1=maskt[:, half:], op=ALU.mult)

            for k in range(NCH):
                sl = slice(k * CHUNK, (k + 1) * CHUNK)
                nc.tensor.matmul(out=psI[:, sl],
                                 lhsT=colT[:, 3 * b:3 * b + 3],
                                 rhs=contrib[:, sl],
                                 start=(b == 0), stop=True,
                                 skip_group_check=(b != 0))

            if b != nb - 1:
                for k in range(NCH):
                    sl = slice(k * CHUNK, (k + 1) * CHUNK)
                    nc.tensor.matmul(out=psS[:, sl], lhsT=lows[:],
                                     rhs=lt[:, sl],
                                     start=False, stop=True, skip_group_check=True)

        imgsb = P.tile([3, NPIX], F32, tag="imgsb", name="imgsb")
        for k in range(NCH):
            sl = slice(k * CHUNK, (k + 1) * CHUNK)
            nc.vector.tensor_copy(out=imgsb[:, sl], in_=psI[:, sl])
        nc.sync.dma_start(img_d[:], imgsb[:])

    nc.compile()
    return nc


def _make_runner(nc, n_cores=NCORES):
    import jax
    from jax.sharding import Mesh, PartitionSpec
    from jax.experimental.shard_map import shard_map

    from concourse import mybir
    from concourse.bass2jax import (_bass_exec_p, install_neuronx_cc_hook,
                                    partition_id_tensor)

    install_neuronx_cc_hook()
    pn = nc.partition_id_tensor.name if nc.partition_id_tensor else None
    in_names, out_names, out_avals, zero_outs = [], [], [], []
    for alloc in nc.m.functions[0].allocations:
        if not isinstance(alloc, mybir.MemoryLocationSet):
            continue
        name = alloc.memorylocations[0].name
        if alloc.kind == "ExternalInput":
            if name != pn:
                in_names.append(name)
        elif alloc.kind == "ExternalOutput":
            shape = tuple(alloc.tensor_shape)
            dtype = mybir.dt.np(alloc.dtype)
            out_names.append(name)
            out_avals.append(jax.core.ShapedArray(shape, dtype))
            zero_outs.append(np.zeros(shape, dtype))
    n_params = len(in_names)
    n_outs = len(out_avals)
    in_all = in_names + out_names + ([pn] if pn else [])
    donate = tuple(range(n_params, n_params + n_outs))

    def _body(*args):
        ops = list(args)
        if pn is not None:
            ops.append(partition_id_tensor())
        return tuple(_bass_exec_p.bind(
            *ops, out_avals=tuple(out_avals), in_names=tuple(in_all),
            out_names=tuple(out_names), lowering_input_output_aliases=(),
            sim_require_finite=True, sim_require_nnan=True, nc=nc))

    mesh = Mesh(np.asarray(jax.devices()[:n_cores]), ("core",))
    fn = jax.jit(
        shard_map(_body, mesh=mesh,
                  in_specs=(PartitionSpec("core"),) * (n_params + n_outs),
                  out_specs=(PartitionSpec("core"),) * len(out_names),
                  check_rep=False),
        donate_argnums=donate, keep_unused=True)

    # The donated output buffers are a PJRT output-binding artifact: the NEFF
    # writes every element of img (verified: a garbage-filled buffer yields a
    # bit-identical image), so their contents never matter. Recycle the
    # previous call's output arrays as the next call's donated buffers --
    # they are already on device, so no zero-buffer ever rides the tunnel.
    from jax.sharding import NamedSharding
    zsharding = NamedSharding(mesh, PartitionSpec("core"))
    state = {"bufs": None}

    def _fresh_bufs():
        return [
            jax.device_put(
                np.zeros((n_cores * z.shape[0], *z.shape[1:]), z.dtype),
                zsharding)
            for z in zero_outs
        ]

    def run(in_maps):
        concat_in = [
            np.concatenate([np.asarray(m[name]) for m in in_maps], axis=0)
            for name in in_names
        ]
        bufs = state["bufs"] if state["bufs"] is not None else _fresh_bufs()
        state["bufs"] = None                # consumed by donation below
        out_arrs = fn(*concat_in, *bufs)
        results = [
            {name: np.asarray(out_arrs[i]).reshape(n_cores, *out_avals[i].shape)[c]
             for i, name in enumerate(out_names)}
            for c in range(n_cores)
        ]
        state["bufs"] = list(out_arrs)      # recycle for the next call
        return results

    return run


def _stage_inputs(points, cov_factor, colors, opacity, extrinsic, fx, fy):
    """Project gaussians on host (f64), depth-sort, cull globally, shard the
    sorted list across cores, pack each shard into one [128, CSH] tensor."""
    N = points.shape[0]
    pts = np.asarray(points, np.float32)
    ex = np.asarray(extrinsic, np.float32)

    # depth order as the reference computes it (f32 matmul; verified
    # bit-identical to the jax cpu matmul the reference uses)
    ph = np.concatenate([pts, np.ones((N, 1), np.float32)], axis=1)
    z32 = (ph @ ex)[:, 2]
    order = np.argsort(z32, kind="stable")

    pc = ph.astype(np.float64) @ ex.astype(np.float64)
    x, y, z = pc[:, 0], pc[:, 1], pc[:, 2]
    zs = np.where(z == 0.0, 1e-30, z)
    cf = np.asarray(cov_factor, np.float64)
    cov3d = 0.05 * np.matmul(cf, cf.transpose(0, 2, 1)) + 1e-4 * np.eye(3)
    Rm = ex[:3, :3].astype(np.float64).T
    J = np.zeros((N, 2, 3))
    J[:, 0, 0] = fx / zs
    J[:, 0, 2] = fx * x / zs**2
    J[:, 1, 1] = fy / zs
    J[:, 1, 2] = fy * y / zs**2
    T = np.matmul(J, Rm)
    cov2d = np.matmul(np.matmul(T, cov3d), T.transpose(0, 2, 1))
    a, b_, c = cov2d[:, 0, 0], cov2d[:, 0, 1], cov2d[:, 1, 1]
    det = a * c - b_ * b_
    inv_det = 1.0 / np.maximum(det, 1e-12)
    m05ia = -0.5 * c * inv_det
    m05ic = -0.5 * a * inv_det
    mib = b_ * inv_det
    mid = 0.5 * (a + c)
    lam = mid + np.sqrt(np.maximum(mid * mid - det, 0.1))
    rad = np.ceil(3.0 * np.sqrt(np.maximum(lam, 0.0)))
    rad = np.nan_to_num(rad, nan=1e9, posinf=1e9)
    tfx = W / (2.0 * fx)
    tfy = H / (2.0 * fy)
    pxp = fx * np.clip(x / zs, -1.3 * tfx, 1.3 * tfx) + 0.5 * W
    pyp = fy * np.clip(y / zs, -1.3 * tfy, 1.3 * tfy) + 0.5 * H
    in_view = (z > ZNEAR) & (det > 0)
    opac = np.asarray(opacity, np.float64)
    lsigm = np.where(in_view, -np.logaddexp(0.0, -opac), -BIGNEG)

    m05ia = np.where(in_view, m05ia, 0.0)
    m05ic = np.where(in_view, m05ic, 0.0)
    mib = np.where(in_view, mib, 0.0)
    pxp = np.where(in_view, pxp, 0.0)
    pyp = np.where(in_view, pyp, 0.0)
    rad = np.where(in_view, rad, -1.0)

    # global cull: drop gaussians invisible to the whole image
    M = 2.0
    kill = (~in_view) | (pxp + rad < -M) | (pxp - rad > W - 1 + M) \
        | (pyp + rad < -M) | (pyp - rad > H - 1 + M)
    keep = order[~kill[order]]
    n = len(keep)
    nb = NCORES * max(1, int(np.ceil(n / (128.0 * NCORES))))
    nbs = nb // NCORES
    CSH = 10 * nbs + ROWS

    cols = np.asarray(colors, np.float32)
    planes = [(pxp, 0.0), (pyp, 0.0), (m05ia, 0.0), (m05ic, 0.0),
              (mib, 0.0), (rad, -1.0), (lsigm, -BIGNEG)]

    # pack the full sorted list block-major, split into per-core shards.
    # shard s holds cols [7 planes x nbs | colors interleaved 3 x nbs]:
    #   plane part: p * nbs + k   (device unpacks "p (t n) -> p t n", t=7)
    #   color part: 3 * k + c     (colT block k at cols 3k..3k+3)
    P7 = np.empty((7, nb * 128), np.float32)
    for p, (arr, padval) in enumerate(planes):
        P7[p, :n] = arr[keep]
        P7[p, n:] = padval
    A = P7.reshape(7, NCORES, nbs, 128).transpose(1, 3, 0, 2)   # [8,128,7,nbs]
    padded = np.zeros((nb * 128, 3), np.float32)
    padded[:n] = cols[keep]
    B = padded.reshape(NCORES, nbs, 128, 3).transpose(0, 2, 1, 3)  # [8,128,nbs,3]

    pk_all = np.empty((NCORES, 128, CSH), np.float32)
    pk_all[:, :, :7 * nbs] = A.reshape(NCORES, 128, 7 * nbs)
    pk_all[:, :, 7 * nbs:10 * nbs] = B.reshape(NCORES, 128, 3 * nbs)
    rowg = np.arange(H, dtype=np.float32).reshape(NCORES, 1, ROWS)
    pk_all[:, :, 10 * nbs:] = rowg
    in_maps = [{"pk": pk_all[cidx]} for cidx in range(NCORES)]

    sig = 1.0 / (1.0 + np.exp(-float(np.asarray(opacity, np.float64).max())))
    use_clamp = bool(sig > 0.985)
    return in_maps, nb, use_clamp


def kernel(points, cov_factor, colors, opacity, extrinsic, focal_x, focal_y,
           width, height):
    fx, fy = float(focal_x), float(focal_y)
    assert int(width) == W and int(height) == H

    in_maps, nb, use_clamp = _stage_inputs(points, cov_factor, colors, opacity,
                                           extrinsic, fx, fy)
    key = (nb, use_clamp)
    if key not in _program_cache:
        nc = _build_program(*key)
        _program_cache[key] = (nc, _make_runner(nc))
    nc, run = _program_cache[key]

    results = run(in_maps)

    out = np.zeros((H, W, 3), np.float32)
    for cidx in range(NCORES):
        band = results[cidx]["img"].reshape(3, ROWS, W)
        out[cidx * ROWS:(cidx + 1) * ROWS] = band.transpose(1, 2, 0)
    return out
